# revision 8
# baseline (speedup 1.0000x reference)
"""Trainium2 Bass kernel for nn_DeltaModel (scatter_memory).

Algorithm: every per-token quantity (embedding -> MLP -> LayerNorm -> k/v/q
projections) is a pure function of the vocab id (V=64), so the encode collapses
to 64-row tables computed once on device.  The delta-rule scan
    M_{t+1} = M_t + (v_t - M_t k_t) k_t^T ,  out = M_T q
collapses (since only M_T @ q is needed) to a backward vector recursion
    u <- q;  for t = T-1..0:  a_t = k_t . u ;  u <- u - a_t k_t
    M_T q = sum_t a_t v_t
On device: k_t / v_t rows are indirect-DMA row-gathers from the tables by seq
ids; the recursion runs on the Vector engine (2 fused ops per step, batch on
partitions); the answer sum runs as per-chunk PE matmuls accumulated in PSUM.
Sign trick: the update is computed as u' = (k*a) - u (hardware op order), which
flips the sign of u every step; the stored a_t alternate sign accordingly and
are corrected by a +/-1 parity scale folded into the answer matmuls.

Sharding: pure data parallel, batch 256 -> 8 cores x 32.
"""

import numpy as np

B, L, V, H = 256, 2048, 64, 64  # problem shape (hardcoded per spec)
NCORES = 8
BL = B // NCORES  # 32
T_FULL = L - 1  # 2047
SUPER = 128  # sweep gather tile (time steps)
CHUNK = 128  # answer-matmul chunk (time steps)

_CACHE = {}
LAST_RESULTS = None


def _strip_same_engine_waits(nc, mybir):
    """Remove semaphore waits that only order instructions within one engine.

    Engines execute their instruction streams in order (the DVE pipeline
    flush between ops is the output-dependency barrier — sem or no sem, the
    next op issues only after the previous op's writes drain), so a wait on a
    semaphore that is updated exclusively by the waiting instruction's own
    engine is redundant.  Cross-engine waits (DMA completion, PE/ACT
    consumers of DVE results, barriers) are kept.
    """
    fn = nc.m.functions[0]
    updaters = {}
    insts = []
    for bb in fn.blocks:
        for inst in bb.instructions:
            insts.append(inst)
            si = getattr(inst, "sync_info", None)
            if si:
                for u in (si.on_update or []):
                    if u.sync_type == "semaphore":
                        updaters.setdefault(u.id, set()).add(inst.engine)
    strippable = {
        mybir.EngineType.DVE,
    }
    ok_types = ("InstTensorScalarPtr", "InstTensorTensor", "InstTensorCopy")
    nstrip = 0
    for inst in insts:
        si = getattr(inst, "sync_info", None)
        if not si or not si.on_wait or inst.engine not in strippable:
            continue
        if type(inst).__name__ not in ok_types:
            continue
        if type(inst).__name__ == "InstTensorScalarPtr" and len(inst.outs) != 2:
            continue  # only dot ops (out + accum_out); updates keep waits
        keep = []
        for w in si.on_wait:
            if (w.sync_type == "semaphore" and w.wait_mode == "sem-ge-imm"
                    and updaters.get(w.id) == {inst.engine}):
                nstrip += 1
                continue
            keep.append(w)
        if len(keep) != len(si.on_wait):
            si.on_wait = keep
    return nstrip


def _build_nc(T):
    import concourse.bass as bass
    import concourse.mybir as mybir
    import concourse.tile as tile
    from concourse import bacc

    f32 = mybir.dt.float32
    i32 = mybir.dt.int32
    Alu = mybir.AluOpType
    Act = mybir.ActivationFunctionType

    nc = bacc.Bacc("TRN2", target_bir_lowering=False, debug=False,
                   num_devices=NCORES)

    # ---- I/O -----------------------------------------------------------
    TP = (T + SUPER - 1) // SUPER * SUPER  # padded step count (2048)
    NST = TP // SUPER
    NCH = TP // CHUNK
    i16 = mybir.dt.int16
    kidx_d = nc.dram_tensor("kidx", [128, NST * SUPER * 8], i16,
                            kind="ExternalInput")
    vidx_d = nc.dram_tensor("vidx", [128, NCH * BL * CHUNK // 16], i16,
                            kind="ExternalInput")
    qidx_d = nc.dram_tensor("qidx", [128, 8], i16, kind="ExternalInput")
    embT_d = nc.dram_tensor("embT", [H + 1, V], f32, kind="ExternalInput")
    w1b1_d = nc.dram_tensor("w1b1", [H + 1, 2 * H], f32, kind="ExternalInput")
    w2T_d = nc.dram_tensor("w2T", [2 * H, H], f32, kind="ExternalInput")
    eb2_d = nc.dram_tensor("eb2", [V, H], f32, kind="ExternalInput")
    wkb_d = nc.dram_tensor("wkb", [H + 1, H], f32, kind="ExternalInput")
    wvb_d = nc.dram_tensor("wvb", [H + 1, H], f32, kind="ExternalInput")
    wqb_d = nc.dram_tensor("wqb", [H + 1, H], f32, kind="ExternalInput")
    wrpb_d = nc.dram_tensor("wrpb", [H + 1, H], f32, kind="ExternalInput")
    woutb_d = nc.dram_tensor("woutb", [H + 1, V], f32, kind="ExternalInput")
    iden_d = nc.dram_tensor("iden", [128, 128], f32, kind="ExternalInput")
    pm_d = nc.dram_tensor("pm", [128, 1], f32, kind="ExternalInput")
    out_d = nc.dram_tensor("out", [BL, V], f32, kind="ExternalOutput")

    kn_d = nc.dram_tensor("kn_scratch", [V, H], f32, kind="Internal")
    vt_d = nc.dram_tensor("vt_scratch", [V, H], f32, kind="Internal")
    qt_d = nc.dram_tensor("qt_scratch", [V, H], f32, kind="Internal")

    with tile.TileContext(nc) as tc:
        with (
            tc.tile_pool(name="const", bufs=1) as cp,
            tc.tile_pool(name="setup", bufs=1) as sp,
            tc.tile_pool(name="setup_ps", bufs=2, space="PSUM") as spp,
            tc.tile_pool(name="sweep", bufs=1) as swp,
            tc.tile_pool(name="kst", bufs=2) as kp,
            tc.tile_pool(name="vst", bufs=3) as vp,
            tc.tile_pool(name="ans_ps", bufs=2, space="PSUM") as ap_pool,
            tc.tile_pool(name="at_ps", bufs=2, space="PSUM") as atp,
        ):
            # ---- load constants ---------------------------------------
            def load(pool, dram, shape, tag, dtype=f32):
                t = pool.tile(shape, dtype, tag=tag)
                nc.gpsimd.dma_start(out=t[:], in_=dram.ap())
                return t

            TPW = NST * SUPER * 8  # kidx free width (num_idxs/16 per st = SUPER*8)
            vidx_sb = load(cp, vidx_d, [128, NCH * BL * CHUNK // 16],
                           "c_vidx", i16)
            qidx_sb = load(cp, qidx_d, [128, 8], "c_qidx", i16)
            embT = load(cp, embT_d, [H + 1, V], "c_embT")
            w1b1 = load(cp, w1b1_d, [H + 1, 2 * H], "c_w1b1")
            w2T = load(cp, w2T_d, [2 * H, H], "c_w2T")
            eb2 = load(cp, eb2_d, [V, H], "c_eb2")
            wkb = load(cp, wkb_d, [H + 1, H], "c_wkb")
            wvb = load(cp, wvb_d, [H + 1, H], "c_wvb")
            wqb = load(cp, wqb_d, [H + 1, H], "c_wqb")
            wrpb = load(cp, wrpb_d, [H + 1, H], "c_wrpb")
            woutb = load(cp, woutb_d, [H + 1, V], "c_woutb")
            iden = load(cp, iden_d, [128, 128], "c_iden")
            pm = load(cp, pm_d, [128, 1], "c_pm")

            # ---- setup: tables ----------------------------------------
            ps1 = spp.tile([V, 2 * H], f32, tag="sps")
            nc.tensor.matmul(ps1[:], lhsT=embT[:], rhs=w1b1[:], start=True,
                             stop=True)
            r1 = sp.tile([V, 2 * H], f32)
            nc.scalar.activation(r1[:], ps1[:], Act.Relu)

            ps2 = spp.tile([2 * H, V], f32, tag="sps")
            nc.tensor.transpose(ps2[:], r1[:], iden[:V, :V])
            r1t = sp.tile([2 * H, V], f32)
            nc.scalar.copy(r1t[:], ps2[:])

            ps3 = spp.tile([V, H], f32, tag="sps")
            nc.tensor.matmul(ps3[:], lhsT=r1t[:], rhs=w2T[:], start=True,
                             stop=True)
            htab = sp.tile([V, H], f32)
            nc.vector.tensor_add(htab[:], ps3[:], eb2[:])

            mu = sp.tile([V, 1], f32)
            nc.vector.tensor_reduce(mu[:], htab[:], axis=mybir.AxisListType.X,
                                    op=Alu.add)
            nc.vector.tensor_scalar_mul(mu[:], mu[:], 1.0 / H)
            xc = sp.tile([V, H], f32)
            nc.vector.tensor_scalar_sub(xc[:], htab[:], mu[:])
            sq = sp.tile([V, H], f32)
            var = sp.tile([V, 1], f32)
            nc.scalar.activation(sq[:], xc[:], Act.Square, accum_out=var[:])
            eps = sp.tile([V, 1], f32)
            nc.vector.memset(eps[:], 1e-5)
            sig = sp.tile([V, 1], f32)
            nc.scalar.activation(sig[:], var[:], Act.Sqrt, bias=eps[:],
                                 scale=1.0 / H)
            rstd = sp.tile([V, 1], f32)
            nc.vector.reciprocal(rstd[:], sig[:])
            xcn = sp.tile([V, H], f32)
            nc.vector.tensor_scalar_mul(xcn[:], xc[:], rstd[:])

            ps4 = spp.tile([H, V], f32, tag="sps")
            nc.tensor.transpose(ps4[:], xcn[:], iden[:V, :V])
            xt = sp.tile([H + 1, V], f32)
            nc.vector.memset(xt[H:H + 1, :], 1.0)
            nc.scalar.copy(xt[:H, :], ps4[:])

            kps = spp.tile([V, H], f32, tag="sps")
            nc.tensor.matmul(kps[:], lhsT=xt[:], rhs=wkb[:], start=True,
                             stop=True)
            ksq = sp.tile([V, H], f32)
            kn2 = sp.tile([V, 1], f32)
            nc.scalar.activation(ksq[:], kps[:], Act.Square, accum_out=kn2[:])
            knm = sp.tile([V, 1], f32)
            nc.scalar.activation(knm[:], kn2[:], Act.Sqrt)
            nc.vector.tensor_scalar_max(knm[:], knm[:], 1e-12)
            kiv = sp.tile([V, 1], f32)
            nc.vector.reciprocal(kiv[:], knm[:])
            kn_sb = sp.tile([V, H], f32)
            nc.vector.tensor_scalar_mul(kn_sb[:], kps[:], kiv[:])
            nc.gpsimd.dma_start(out=kn_d.ap(), in_=kn_sb[:])

            vps = spp.tile([V, H], f32, tag="sps")
            nc.tensor.matmul(vps[:], lhsT=xt[:], rhs=wvb[:], start=True,
                             stop=True)
            vt_sb = sp.tile([V, H], f32)
            nc.scalar.copy(vt_sb[:], vps[:])
            nc.gpsimd.dma_start(out=vt_d.ap(), in_=vt_sb[:])

            qps = spp.tile([V, H], f32, tag="sps")
            nc.tensor.matmul(qps[:], lhsT=xt[:], rhs=wqb[:], start=True,
                             stop=True)
            qt_sb = sp.tile([V, H], f32)
            nc.scalar.copy(qt_sb[:], qps[:])
            nc.gpsimd.dma_start(out=qt_d.ap(), in_=qt_sb[:])

            # gather per-batch q rows (lanes 0..BL-1 real, rest dummy)
            qg = sp.tile([128, 1, H], f32)
            nc.gpsimd.dma_gather(
                out_ap=qg[:], in_ap=qt_d.ap(), idxs_ap=qidx_sb[:],
                num_idxs=128, num_idxs_reg=128, elem_size=H)

            # ---- main sweep -------------------------------------------
            u = swp.tile([BL, H], f32)
            nc.vector.tensor_copy(u[:], qg[:BL, 0, :])
            tmp = swp.tile([BL, H], f32)
            alpha = swp.tile([BL, (T + 127) // 128 * 128], f32)
            ans_acc = swp.tile([H, BL], f32)
            nc.vector.memset(ans_acc[:], 0.0)

            # only the padded tail [T:TP) is read before being written (by the
            # final chunk's transpose); the rest is filled by accum_out.
            TPAD = (T + 127) // 128 * 128
            if TPAD > T:
                nc.vector.memset(alpha[:, T:TPAD], 0.0)
            for st in range(NST):
                t0 = st * SUPER
                sc = min(SUPER, T - t0)
                kix = kp.tile([128, SUPER * 8], i16, tag="kix")
                nc.gpsimd.dma_start(
                    out=kix[:], in_=kidx_d.ap()[:, st * SUPER * 8:
                                                (st + 1) * SUPER * 8])
                kst = kp.tile([128, SUPER, H], f32, tag="kst")
                # HW SWDGE caps one gather at ~1024 idxs (65 descriptors)
                npc = SUPER * 128 // 1024
                for piece in range(npc):
                    sl = SUPER // npc
                    nc.gpsimd.dma_gather(
                        out_ap=kst[:, piece * sl:(piece + 1) * sl, :],
                        in_ap=kn_d.ap(),
                        idxs_ap=kix[:, piece * 64:(piece + 1) * 64],
                        num_idxs=1024, num_idxs_reg=1024, elem_size=H)
                for j in range(sc):
                    tau = t0 + j
                    nc.vector.scalar_tensor_tensor(
                        out=tmp[:], in0=u[:], scalar=1.0, in1=kst[:BL, j, :],
                        op0=Alu.mult, op1=Alu.mult,
                        accum_out=alpha[:, tau:tau + 1])
                    nc.vector.scalar_tensor_tensor(
                        out=u[:], in0=kst[:BL, j, :],
                        scalar=alpha[:, tau:tau + 1], in1=u[:],
                        op0=Alu.mult, op1=Alu.subtract)
                # answer chunks of this supertile (full CHUNK frames; alpha
                # is zero-padded past T so junk v rows contribute nothing)
                for c0 in range(0, SUPER, CHUNK):
                    tau0 = t0 + c0
                    ci = tau0 // CHUNK
                    vst = vp.tile([CHUNK, BL, H], f32, tag="vst")
                    vbase = ci * BL * CHUNK // 16
                    for piece in range(BL * CHUNK // 1024):
                        nc.gpsimd.dma_gather(
                            out_ap=vst[:, piece * 8:(piece + 1) * 8, :],
                            in_ap=vt_d.ap(),
                            idxs_ap=vidx_sb[:, vbase + piece * 64:
                                            vbase + (piece + 1) * 64],
                            num_idxs=1024, num_idxs_reg=1024, elem_size=H)
                    at_ps = atp.tile([CHUNK, BL], f32)
                    nc.tensor.transpose(at_ps[:],
                                        alpha[:, tau0:tau0 + CHUNK],
                                        iden[:BL, :BL])
                    atb = vp.tile([CHUNK, BL], f32, tag="atb")
                    nc.scalar.mul(atb[:], at_ps[:], pm[:])
                    cps = ap_pool.tile([H, BL], f32, tag="cps")
                    for b in range(BL):
                        nc.tensor.matmul(cps[:, b:b + 1],
                                         lhsT=vst[:, b, :],
                                         rhs=atb[:, b:b + 1],
                                         start=True, stop=True)
                    nc.vector.tensor_add(ans_acc[:], ans_acc[:], cps[:])

            # ---- epilogue ---------------------------------------------
            ansx = sp.tile([H + 1, BL], f32)
            nc.vector.memset(ansx[H:H + 1, :], 1.0)
            nc.scalar.copy(ansx[:H, :], ans_acc[:])
            rps = spp.tile([H, BL], f32, tag="sps")
            nc.tensor.matmul(rps[:], lhsT=wrpb[:], rhs=ansx[:], start=True,
                             stop=True)
            rx = sp.tile([H + 1, BL], f32)
            nc.vector.memset(rx[H:H + 1, :], 1.0)
            nc.scalar.copy(rx[:H, :], rps[:])
            ops_ = spp.tile([V, BL], f32, tag="sps")
            nc.tensor.matmul(ops_[:], lhsT=woutb[:], rhs=rx[:], start=True,
                             stop=True)
            o_sb = sp.tile([V, BL], f32)
            nc.scalar.copy(o_sb[:], ops_[:])
            ot_ps = spp.tile([BL, V], f32, tag="sps")
            nc.tensor.transpose(ot_ps[:], o_sb[:], iden[:V, :V])
            o_fin = sp.tile([BL, V], f32)
            nc.scalar.copy(o_fin[:], ot_ps[:])
            nc.gpsimd.dma_start(out=out_d.ap(), in_=o_fin[:])

    nc.compile()
    return nc


def _marshal(inputs, T):
    f = np.float32
    seq = np.asarray(inputs["seq"])
    embed = np.asarray(inputs["embed"], f)
    W1 = np.asarray(inputs["W1"], f)
    b1 = np.asarray(inputs["b1"], f)
    W2 = np.asarray(inputs["W2"], f)
    b2 = np.asarray(inputs["b2"], f)
    gamma = np.asarray(inputs["gamma"], f)
    beta = np.asarray(inputs["beta"], f)
    Wk = np.asarray(inputs["Wk"], f)
    Wv = np.asarray(inputs["Wv"], f)
    Wq = np.asarray(inputs["Wq"], f)
    Wrp = np.asarray(inputs["Wrp"], f)
    brp = np.asarray(inputs["brp"], f)
    Wout = np.asarray(inputs["Wout"], f)
    bout = np.asarray(inputs["bout"], f)

    ones = np.ones((1,), f)
    shared = {
        "embT": np.vstack([embed.T, np.ones((1, V), f)]).astype(f),
        "w1b1": np.vstack([W1.T, b1[None]]).astype(f),
        "w2T": np.ascontiguousarray(W2.T, f),
        "eb2": (embed + b2[None]).astype(f),
        "wkb": np.vstack([(Wk * gamma[None]).T, (Wk @ beta)[None]]).astype(f),
        "wvb": np.vstack([(Wv * gamma[None]).T, (Wv @ beta)[None]]).astype(f),
        "wqb": np.vstack([(Wq * gamma[None]).T, (Wq @ beta)[None]]).astype(f),
        "wrpb": np.vstack([Wrp.T, brp[None]]).astype(f),
        "woutb": np.vstack([Wout.T, bout[None]]).astype(f),
        "iden": np.eye(128, dtype=f),
        "pm": np.where(np.arange(128) % 2 == 0, 1.0, -1.0).astype(f)[:, None],
    }
    TP = (T + SUPER - 1) // SUPER * SUPER
    NST = TP // SUPER
    NCH = TP // CHUNK

    def wrap(flat):
        n = flat.size
        w16 = np.ascontiguousarray(flat.reshape(n // 16, 16).T).astype(np.int16)
        return np.tile(w16, (8, 1))

    in_maps = []
    for c in range(NCORES):
        sl = slice(c * BL, (c + 1) * BL)
        sseq = seq[sl]
        # reversed-time ids: ids[b, tau] = seq[b, (T-1) - tau]
        ids = np.ascontiguousarray(sseq[:, T - 1::-1]).astype(np.int64)
        idsp = np.zeros((BL, TP), np.int64)
        idsp[:, :T] = ids
        # k-stream: i = slot*128 + p ; p<BL -> ids[p, t0+slot], else dummy 0
        kblocks = []
        for st in range(NST):
            blk = np.zeros((SUPER, 128), np.int64)
            blk[:, :BL] = idsp[:, st * SUPER:(st + 1) * SUPER].T
            kblocks.append(wrap(blk.reshape(-1)))
        # v-stream: i = b*128 + tau ; chunk frames of CHUNK
        vblocks = []
        for ci in range(NCH):
            blk = idsp[:, ci * CHUNK:(ci + 1) * CHUNK]  # [BL, CHUNK]
            vblocks.append(wrap(blk.reshape(-1)))
        qflat = np.zeros(128, np.int64)
        qflat[:BL] = sseq[:, L - 1]
        m = dict(shared)
        m["kidx"] = np.concatenate(kblocks, axis=1)
        m["vidx"] = np.concatenate(vblocks, axis=1)
        m["qidx"] = wrap(qflat)
        in_maps.append(m)
    return in_maps


def kernel(**inputs):
    global LAST_RESULTS
    import os
    from concourse.bass_utils import run_bass_kernel_spmd

    T = T_FULL
    if "nc" not in _CACHE:
        _CACHE["nc"] = _build_nc(T)
    nc = _CACHE["nc"]
    in_maps = _marshal(inputs, T)
    trace = bool(int(os.environ.get("KERNEL_TRACE", "0")))
    res = run_bass_kernel_spmd(nc, in_maps, core_ids=list(range(NCORES)),
                               trace=trace)
    LAST_RESULTS = res
    out = np.concatenate([res.results[c]["out"] for c in range(NCORES)],
                         axis=0)
    return out.astype(np.float32)



# revision 11
# speedup vs baseline: 1.2896x; 1.2896x over previous
"""Trainium2 Bass kernel for nn_DeltaModel — windowed PE-dot variant.

Same backward-recursion algorithm as the baseline (see kernel.py docstring),
but the per-step DVE dot is replaced by per-window PE matvecs computed against
a one-window-stale u snapshot, plus a narrow DVE correction op per step:

  window m (S=8 steps), snapshot u_snap = u after window m-2:
    d_j   = k_j . u_snap                       (PE, 32 per-batch matvecs, fp32)
    beta_j = sum(gam16_row * [beta_hist; -d])  (DVE STT mult-mult accum,
                                                width 16; -d pre-written into
                                                the alpha slab by ACT)
    u    += beta_j k_j                         (DVE STT, as baseline update)

beta = -alpha throughout; the answer matmuls fold the sign via pm = -1.
Gam-coefficient streams (history window, zero-masked at epoch boundaries) are
marshalled host-side from the token tables. K^T columns for the PE dots come
from a plain row-gather (slot = (b%16)*8+j, slab = wi*2+b//16) followed by one
PE transpose + ACT copy per 16-batch slab per window — dma_gather's
transpose=True mode is broken on this NEFF path and must not be used.
"""

import numpy as np

B, L, V, H = 256, 2048, 64, 64
NCORES = 8
BL = B // NCORES  # 32
T_FULL = L - 1  # 2047
SUPER = 128  # sweep tile (time steps)
CHUNK = 128  # answer-matmul chunk (time steps)
S = 8        # window (PE dot block)
WH = 2 * S      # correction window width (15 hist + d slot)

_CACHE = {}
LAST_RESULTS = None


def _build_nc(T):
    import concourse.bass as bass
    import concourse.mybir as mybir
    import concourse.tile as tile
    from concourse import bacc

    f32 = mybir.dt.float32
    f16 = mybir.dt.float16
    i16 = mybir.dt.int16
    Alu = mybir.AluOpType
    Act = mybir.ActivationFunctionType

    nc = bacc.Bacc("TRN2", target_bir_lowering=False, debug=False,
                   num_devices=NCORES)

    TP = (T + SUPER - 1) // SUPER * SUPER  # 2048
    NST = TP // SUPER
    NCH = TP // CHUNK
    NWIN = TP // S

    kidx_d = nc.dram_tensor("kidx", [128, NST * SUPER * 8], i16,
                            kind="ExternalInput")
    vidx_d = nc.dram_tensor("vidx", [128, NCH * BL * CHUNK // 16], i16,
                            kind="ExternalInput")
    qidx_d = nc.dram_tensor("qidx", [128, 8], i16, kind="ExternalInput")
    ktidx_d = nc.dram_tensor("ktidx", [128, NST * 256], i16,
                             kind="ExternalInput")
    gam_d = nc.dram_tensor("gam", [BL, TP * WH], f32, kind="ExternalInput")
    embT_d = nc.dram_tensor("embT", [H + 1, V], f32, kind="ExternalInput")
    w1b1_d = nc.dram_tensor("w1b1", [H + 1, 2 * H], f32, kind="ExternalInput")
    w2T_d = nc.dram_tensor("w2T", [2 * H, H], f32, kind="ExternalInput")
    eb2_d = nc.dram_tensor("eb2", [V, H], f32, kind="ExternalInput")
    wkb_d = nc.dram_tensor("wkb", [H + 1, H], f32, kind="ExternalInput")
    wvb_d = nc.dram_tensor("wvb", [H + 1, H], f32, kind="ExternalInput")
    wqb_d = nc.dram_tensor("wqb", [H + 1, H], f32, kind="ExternalInput")
    wrpb_d = nc.dram_tensor("wrpb", [H + 1, H], f32, kind="ExternalInput")
    woutb_d = nc.dram_tensor("woutb", [H + 1, V], f32, kind="ExternalInput")
    iden_d = nc.dram_tensor("iden", [128, 128], f32, kind="ExternalInput")
    pm_d = nc.dram_tensor("pm", [128, 1], f32, kind="ExternalInput")
    out_d = nc.dram_tensor("out", [BL, V], f32, kind="ExternalOutput")

    kn_d = nc.dram_tensor("kn_scratch", [V, H], f32, kind="Internal")
    vt_d = nc.dram_tensor("vt_scratch", [V, H], f32, kind="Internal")
    qt_d = nc.dram_tensor("qt_scratch", [V, H], f32, kind="Internal")

    with tile.TileContext(nc) as tc:
        with (
            tc.tile_pool(name="const", bufs=1) as cp,
            tc.tile_pool(name="setup", bufs=1) as sp,
            tc.tile_pool(name="setup_ps", bufs=1, space="PSUM") as spp,
            tc.tile_pool(name="sweep", bufs=1) as swp,
            tc.tile_pool(name="kst", bufs=2) as kp,
            tc.tile_pool(name="ktp", bufs=2) as ktp,
            tc.tile_pool(name="gmp", bufs=2) as gmp,
            tc.tile_pool(name="vst", bufs=3) as vp,
            tc.tile_pool(name="win", bufs=3) as wp,
            tc.tile_pool(name="win_ps", bufs=3, space="PSUM") as wpp,
            tc.tile_pool(name="ans_ps", bufs=2, space="PSUM") as ap_pool,
            tc.tile_pool(name="at_ps", bufs=2, space="PSUM") as atp,
        ):
            def load(pool, dram, shape, tag, dtype=f32):
                t = pool.tile(shape, dtype, tag=tag, name=tag)
                nc.gpsimd.dma_start(out=t[:], in_=dram.ap())
                return t

            vidx_sb = load(cp, vidx_d, [128, NCH * BL * CHUNK // 16],
                           "c_vidx", i16)
            qidx_sb = load(cp, qidx_d, [128, 8], "c_qidx", i16)
            ktidx_sb = load(cp, ktidx_d, [128, NST * 256], "c_ktidx", i16)
            embT = load(cp, embT_d, [H + 1, V], "c_embT")
            w1b1 = load(cp, w1b1_d, [H + 1, 2 * H], "c_w1b1")
            w2T = load(cp, w2T_d, [2 * H, H], "c_w2T")
            eb2 = load(cp, eb2_d, [V, H], "c_eb2")
            wkb = load(cp, wkb_d, [H + 1, H], "c_wkb")
            wvb = load(cp, wvb_d, [H + 1, H], "c_wvb")
            wqb = load(cp, wqb_d, [H + 1, H], "c_wqb")
            wrpb = load(cp, wrpb_d, [H + 1, H], "c_wrpb")
            woutb = load(cp, woutb_d, [H + 1, V], "c_woutb")
            iden = load(cp, iden_d, [128, 128], "c_iden")
            pm = load(cp, pm_d, [128, 1], "c_pm")

            # ---- setup: tables (same as baseline) ----------------------
            ps1 = spp.tile([V, 2 * H], f32, tag="sps")
            nc.tensor.matmul(ps1[:], lhsT=embT[:], rhs=w1b1[:], start=True,
                             stop=True)
            r1 = sp.tile([V, 2 * H], f32)
            nc.scalar.activation(r1[:], ps1[:], Act.Relu)

            ps2 = spp.tile([2 * H, V], f32, tag="sps")
            nc.tensor.transpose(ps2[:], r1[:], iden[:V, :V])
            r1t = sp.tile([2 * H, V], f32)
            nc.scalar.copy(r1t[:], ps2[:])

            ps3 = spp.tile([V, H], f32, tag="sps")
            nc.tensor.matmul(ps3[:], lhsT=r1t[:], rhs=w2T[:], start=True,
                             stop=True)
            htab = sp.tile([V, H], f32)
            nc.vector.tensor_add(htab[:], ps3[:], eb2[:])

            mu = sp.tile([V, 1], f32)
            nc.vector.tensor_reduce(mu[:], htab[:], axis=mybir.AxisListType.X,
                                    op=Alu.add)
            nc.vector.tensor_scalar_mul(mu[:], mu[:], 1.0 / H)
            xc = sp.tile([V, H], f32)
            nc.vector.tensor_scalar_sub(xc[:], htab[:], mu[:])
            sq = sp.tile([V, H], f32)
            var = sp.tile([V, 1], f32)
            nc.scalar.activation(sq[:], xc[:], Act.Square, accum_out=var[:])
            eps = sp.tile([V, 1], f32)
            nc.vector.memset(eps[:], 1e-5)
            sig = sp.tile([V, 1], f32)
            nc.scalar.activation(sig[:], var[:], Act.Sqrt, bias=eps[:],
                                 scale=1.0 / H)
            rstd = sp.tile([V, 1], f32)
            nc.vector.reciprocal(rstd[:], sig[:])
            xcn = sp.tile([V, H], f32)
            nc.vector.tensor_scalar_mul(xcn[:], xc[:], rstd[:])

            ps4 = spp.tile([H, V], f32, tag="sps")
            nc.tensor.transpose(ps4[:], xcn[:], iden[:V, :V])
            xt = sp.tile([H + 1, V], f32)
            nc.vector.memset(xt[H:H + 1, :], 1.0)
            nc.scalar.copy(xt[:H, :], ps4[:])

            kps = spp.tile([V, H], f32, tag="sps")
            nc.tensor.matmul(kps[:], lhsT=xt[:], rhs=wkb[:], start=True,
                             stop=True)
            ksq = sp.tile([V, H], f32)
            kn2 = sp.tile([V, 1], f32)
            nc.scalar.activation(ksq[:], kps[:], Act.Square, accum_out=kn2[:])
            knm = sp.tile([V, 1], f32)
            nc.scalar.activation(knm[:], kn2[:], Act.Sqrt)
            nc.vector.tensor_scalar_max(knm[:], knm[:], 1e-12)
            kiv = sp.tile([V, 1], f32)
            nc.vector.reciprocal(kiv[:], knm[:])
            kn_sb = sp.tile([V, H], f32)
            nc.vector.tensor_scalar_mul(kn_sb[:], kps[:], kiv[:])
            nc.gpsimd.dma_start(out=kn_d.ap(), in_=kn_sb[:])


            vps = spp.tile([V, H], f32, tag="sps")
            nc.tensor.matmul(vps[:], lhsT=xt[:], rhs=wvb[:], start=True,
                             stop=True)
            vt_sb = sp.tile([V, H], f32)
            nc.scalar.copy(vt_sb[:], vps[:])
            nc.gpsimd.dma_start(out=vt_d.ap(), in_=vt_sb[:])

            qps = spp.tile([V, H], f32, tag="sps")
            nc.tensor.matmul(qps[:], lhsT=xt[:], rhs=wqb[:], start=True,
                             stop=True)
            qt_sb = sp.tile([V, H], f32)
            nc.scalar.copy(qt_sb[:], qps[:])
            nc.gpsimd.dma_start(out=qt_d.ap(), in_=qt_sb[:])

            qg = sp.tile([128, 1, H], f32)
            nc.gpsimd.dma_gather(
                out_ap=qg[:], in_ap=qt_d.ap(), idxs_ap=qidx_sb[:],
                num_idxs=128, num_idxs_reg=128, elem_size=H)

            # ---- main sweep -------------------------------------------
            u = swp.tile([BL, H], f32)
            nc.vector.tensor_copy(u[:], qg[:BL, 0, :])
            alpha = swp.tile([BL, 16 + TP], f32)
            nc.vector.memset(alpha[:, :16], 0.0)
            ans_acc = swp.tile([H, BL], f32)
            nc.vector.memset(ans_acc[:], 0.0)

            # initial snapshot (u = q), used by windows 0 and 1
            u16_list = {}
            ucp0 = wp.tile([BL, H], f32, tag="ucp", name="ucp0")
            nc.vector.tensor_copy(ucp0[:], u[:])
            utp0 = wpp.tile([H, BL], f32, tag="wps", name="utp0")
            nc.tensor.transpose(utp0[:], ucp0[:], iden[:BL, :BL])
            u16_0 = wp.tile([H, BL], f32, tag="u16", name="u16_0")
            nc.scalar.copy(u16_0[:], utp0[:])
            u16_list[0] = u16_0

            NWPS = SUPER // S  # windows per supertile
            for st in range(NST):
                t0 = st * SUPER
                kix = kp.tile([128, SUPER * 8], i16, tag="kix")
                nc.gpsimd.dma_start(
                    out=kix[:], in_=kidx_d.ap()[:, st * SUPER * 8:
                                                (st + 1) * SUPER * 8])
                kst = kp.tile([128, SUPER, H], f32, tag="kst")
                npc = SUPER * 128 // 1024
                for piece in range(npc):
                    sl = SUPER // npc
                    nc.gpsimd.dma_gather(
                        out_ap=kst[:, piece * sl:(piece + 1) * sl, :],
                        in_ap=kn_d.ap(),
                        idxs_ap=kix[:, piece * 64:(piece + 1) * 64],
                        num_idxs=1024, num_idxs_reg=1024, elem_size=H)
                # key rows for PE dots: slot = (b%16)*8 + j, slab = wi*2 + b//16
                kst2 = ktp.tile([128, 2 * (SUPER // S), H], f32, tag="kt")
                for piece in range(4):
                    nc.gpsimd.dma_gather(
                        out_ap=kst2[:, piece * 8:(piece + 1) * 8, :],
                        in_ap=kn_d.ap(),
                        idxs_ap=ktidx_sb[:, st * 256 + piece * 64:
                                         st * 256 + (piece + 1) * 64],
                        num_idxs=1024, num_idxs_reg=1024, elem_size=H)
                gam = gmp.tile([BL, SUPER * WH], f32, tag="gam")
                nc.gpsimd.dma_start(
                    out=gam[:], in_=gam_d.ap()[:, st * SUPER * WH:
                                               (st + 1) * SUPER * WH])

                for wi in range(NWPS):
                    m = st * NWPS + wi
                    u16 = u16_list[m]
                    # PE dots first (inputs ready early; keeps PE ahead):
                    # transpose the 2 key slabs of this window to [H, 128]
                    kts = []
                    for s1 in range(2):
                        ktp_ps = wpp.tile([H, 128], f32, tag="wps",
                                          name=f"ktp{m}_{s1}")
                        nc.tensor.transpose(ktp_ps[:],
                                            kst2[:, 2 * wi + s1, :],
                                            iden[:, :])
                        kts_sb = wp.tile([H, 128], f32, tag=f"kts{s1}",
                                         name=f"kts{m}_{s1}")
                        nc.scalar.copy(kts_sb[:], ktp_ps[:])
                        kts.append(kts_sb)
                    # dps[j, b] = k_{t0+wi*S+j}^b . u_snap_b
                    dps = wpp.tile([S, BL], f32, tag="wps", name=f"dps{m}")
                    for b in range(BL):
                        nc.tensor.matmul(
                            dps[:, b:b + 1],
                            lhsT=kts[b // 16][:, (b % 16) * S:
                                              (b % 16) * S + S],
                            rhs=u16[:, b:b + 1],
                            start=True, stop=True)
                    dsbt = wp.tile([S, BL], f32, tag="dsbt", name=f"dt{m}")
                    nc.scalar.copy(dsbt[:], dps[:])
                    dtp = wpp.tile([BL, S], f32, tag="wps", name=f"dtp{m}")
                    nc.tensor.transpose(dtp[:], dsbt[:], iden[:S, :S])
                    # negate via pm (= -1) and write -d directly into
                    # the alpha slab (slot 16+tau); corr overwrites in place
                    nc.scalar.mul(alpha[:, 16 + t0 + wi * S:
                                        16 + t0 + wi * S + S], dtp[:], pm[:BL])
                    # snapshot for window m+1's dots (u after window m-1,
                    # captured before window m's updates)
                    if m + 1 < NWIN:
                        ucp = wp.tile([BL, H], f32, tag="ucp",
                                      name=f"ucp{m + 1}")
                        nc.vector.tensor_copy(ucp[:], u[:])
                        utp = wpp.tile([H, BL], f32, tag="wps",
                                       name=f"utp{m + 1}")
                        nc.tensor.transpose(utp[:], ucp[:], iden[:BL, :BL])
                        u16n = wp.tile([H, BL], f32, tag="u16",
                                       name=f"u16_{m + 1}")
                        nc.scalar.copy(u16n[:], utp[:])
                        u16_list[m + 1] = u16n

                    jnk = wp.tile([BL, WH], f32, tag="jnk", name=f"jk{m}")

                    def corr(j):
                        tau = t0 + wi * S + j
                        jj = wi * S + j
                        # beta_tau = sum(gam16 * [beta_hist; -d]) — coeff +1
                        # at the d slot (alpha[16+tau], written by the ACT
                        # copy above); accum overwrites that slot in place
                        nc.vector.scalar_tensor_tensor(
                            out=jnk[:],
                            in0=gam[:, jj * WH:(jj + 1) * WH],
                            scalar=1.0,
                            in1=alpha[:, tau + 1:tau + 17],
                            op0=Alu.mult, op1=Alu.mult,
                            accum_out=alpha[:, 16 + tau:17 + tau])

                    def upd(j):
                        tau = t0 + wi * S + j
                        jj = wi * S + j
                        # u += beta * k   (true u maintained)
                        nc.vector.scalar_tensor_tensor(
                            out=u[:], in0=kst[:BL, jj, :],
                            scalar=alpha[:, 16 + tau:17 + tau], in1=u[:],
                            op0=Alu.mult, op1=Alu.add)

                    # software-pipelined: upd lags corr by one step so the
                    # engine executes upd(j-1) while corr(j)'s accumulator
                    # side-effects propagate (hides ~155ns/step)
                    for j in range(S):
                        corr(j)
                        if j > 0:
                            upd(j - 1)
                    upd(S - 1)

                # answer chunks (pm = -1 fixes the beta sign)
                for c0 in range(0, SUPER, CHUNK):
                    tau0 = t0 + c0
                    ci = tau0 // CHUNK
                    vst = vp.tile([CHUNK, BL, H], f32, tag="vst")
                    vbase = ci * BL * CHUNK // 16
                    for piece in range(BL * CHUNK // 1024):
                        nc.gpsimd.dma_gather(
                            out_ap=vst[:, piece * 8:(piece + 1) * 8, :],
                            in_ap=vt_d.ap(),
                            idxs_ap=vidx_sb[:, vbase + piece * 64:
                                            vbase + (piece + 1) * 64],
                            num_idxs=1024, num_idxs_reg=1024, elem_size=H)
                    at_ps = atp.tile([CHUNK, BL], f32)
                    nc.tensor.transpose(at_ps[:],
                                        alpha[:, 16 + tau0:16 + tau0 + CHUNK],
                                        iden[:BL, :BL])
                    atb = vp.tile([CHUNK, BL], f32, tag="atb")
                    nc.scalar.mul(atb[:], at_ps[:], pm[:])
                    cps = ap_pool.tile([H, BL], f32, tag="cps")
                    for b in range(BL):
                        nc.tensor.matmul(cps[:, b:b + 1],
                                         lhsT=vst[:, b, :],
                                         rhs=atb[:, b:b + 1],
                                         start=True, stop=True)
                    nc.vector.tensor_add(ans_acc[:], ans_acc[:], cps[:])

            # ---- epilogue (unchanged) ---------------------------------
            ansx = sp.tile([H + 1, BL], f32)
            nc.vector.memset(ansx[H:H + 1, :], 1.0)
            nc.scalar.copy(ansx[:H, :], ans_acc[:])
            rps = spp.tile([H, BL], f32, tag="sps")
            nc.tensor.matmul(rps[:], lhsT=wrpb[:], rhs=ansx[:], start=True,
                             stop=True)
            rx = sp.tile([H + 1, BL], f32)
            nc.vector.memset(rx[H:H + 1, :], 1.0)
            nc.scalar.copy(rx[:H, :], rps[:])
            ops_ = spp.tile([V, BL], f32, tag="sps")
            nc.tensor.matmul(ops_[:], lhsT=woutb[:], rhs=rx[:], start=True,
                             stop=True)
            o_sb = sp.tile([V, BL], f32)
            nc.scalar.copy(o_sb[:], ops_[:])
            ot_ps = spp.tile([BL, V], f32, tag="sps")
            nc.tensor.transpose(ot_ps[:], o_sb[:], iden[:V, :V])
            o_fin = sp.tile([BL, V], f32)
            nc.scalar.copy(o_fin[:], ot_ps[:])
            nc.gpsimd.dma_start(out=out_d.ap(), in_=o_fin[:])

    nc.compile()
    return nc


def _marshal(inputs, T):
    f = np.float32
    seq = np.asarray(inputs["seq"])
    embed = np.asarray(inputs["embed"], f)
    W1 = np.asarray(inputs["W1"], f)
    b1 = np.asarray(inputs["b1"], f)
    W2 = np.asarray(inputs["W2"], f)
    b2 = np.asarray(inputs["b2"], f)
    gamma = np.asarray(inputs["gamma"], f)
    beta = np.asarray(inputs["beta"], f)
    Wk = np.asarray(inputs["Wk"], f)
    Wv = np.asarray(inputs["Wv"], f)
    Wq = np.asarray(inputs["Wq"], f)
    Wrp = np.asarray(inputs["Wrp"], f)
    brp = np.asarray(inputs["brp"], f)
    Wout = np.asarray(inputs["Wout"], f)
    bout = np.asarray(inputs["bout"], f)

    # host token tables (match device setup math; used for the Gam stream)
    ff = np.maximum(embed @ W1.T + b1, 0) @ W2.T + b2
    h = embed + ff
    mu = h.mean(-1, keepdims=True)
    var = ((h - mu) ** 2).mean(-1, keepdims=True)
    hs = ((h - mu) / np.sqrt(var + 1e-5) * gamma + beta).astype(f)
    kn = hs @ Wk.T
    kn = (kn / np.maximum(np.linalg.norm(kn, axis=-1, keepdims=True),
                          1e-12)).astype(f)
    Gam = (kn @ kn.T).astype(f)

    shared = {
        "embT": np.vstack([embed.T, np.ones((1, V), f)]).astype(f),
        "w1b1": np.vstack([W1.T, b1[None]]).astype(f),
        "w2T": np.ascontiguousarray(W2.T, f),
        "eb2": (embed + b2[None]).astype(f),
        "wkb": np.vstack([(Wk * gamma[None]).T, (Wk @ beta)[None]]).astype(f),
        "wvb": np.vstack([(Wv * gamma[None]).T, (Wv @ beta)[None]]).astype(f),
        "wqb": np.vstack([(Wq * gamma[None]).T, (Wq @ beta)[None]]).astype(f),
        "wrpb": np.vstack([Wrp.T, brp[None]]).astype(f),
        "woutb": np.vstack([Wout.T, bout[None]]).astype(f),
        "iden": np.eye(128, dtype=f),
        "pm": -np.ones((128, 1), f),
    }
    TP = (T + SUPER - 1) // SUPER * SUPER
    NST = TP // SUPER
    NCH = TP // CHUNK

    def wrap(flat):
        n = flat.size
        w16 = np.ascontiguousarray(flat.reshape(n // 16, 16).T).astype(np.int16)
        return np.tile(w16, (8, 1))

    tau_all = np.arange(TP)
    in_maps = []
    for c in range(NCORES):
        sl = slice(c * BL, (c + 1) * BL)
        sseq = seq[sl]
        ids = np.ascontiguousarray(sseq[:, T - 1::-1]).astype(np.int64)
        idsp = np.full((BL, TP), -1, np.int64)
        idsp[:, :T] = ids
        ids0 = np.where(idsp < 0, 0, idsp)
        # k-row stream (updates): i = slot*128 + p ; p<BL -> ids[p, t0+slot]
        kblocks = []
        for st in range(NST):
            blk = np.zeros((SUPER, 128), np.int64)
            blk[:, :BL] = ids0[:, st * SUPER:(st + 1) * SUPER].T
            kblocks.append(wrap(blk.reshape(-1)))
        # transposed key stream (dots): col = wi*256 + b*8 + j, pad -> V
        ktblocks = []
        for st in range(NST):
            cols = np.zeros(SUPER * BL, np.int64)
            for wi in range(SUPER // S):
                blkw = ids0[:, st * SUPER + wi * S:
                            st * SUPER + wi * S + S]  # [BL, S]
                # n = wi*256 + (b//16)*128 + (b%16)*8 + j
                arr = blkw.reshape(2, 16, S)  # [b//16, b%16, j]
                cols[wi * 256:(wi + 1) * 256] = arr.reshape(-1)
            ktblocks.append(wrap(cols))
        # Gam coefficient stream: entry (b, tau*WH + t): lag = WH - t
        gamarr = np.zeros((BL, TP, WH), f)
        gamarr[:, :, WH - 1] = (tau_all < T).astype(f)[None, :]
        for lag in range(1, WH):
            t = WH - 1 - lag
            i_src = tau_all - lag
            epoch = np.maximum((tau_all // S - 1) * S, 0)
            valid = (i_src >= epoch) & (i_src >= 0) & (tau_all < T)
            vi = np.where(valid)[0]
            if vi.size:
                wt = idsp[:, vi]
                wis = idsp[:, vi - lag]
                ok = (wt >= 0) & (wis >= 0)
                vals = np.where(ok, -Gam[np.maximum(wt, 0),
                                         np.maximum(wis, 0)], 0.0)
                gamarr[:, vi, t] = vals.astype(f)
        vblocks = []
        for ci in range(NCH):
            blk = ids0[:, ci * CHUNK:(ci + 1) * CHUNK]
            vblocks.append(wrap(blk.reshape(-1)))
        qflat = np.zeros(128, np.int64)
        qflat[:BL] = sseq[:, L - 1]
        m = dict(shared)
        m["kidx"] = np.concatenate(kblocks, axis=1)
        m["ktidx"] = np.concatenate(ktblocks, axis=1)
        m["vidx"] = np.concatenate(vblocks, axis=1)
        m["qidx"] = wrap(qflat)
        m["gam"] = np.ascontiguousarray(gamarr.reshape(BL, TP * WH))
        in_maps.append(m)
    return in_maps


def kernel(**inputs):
    global LAST_RESULTS
    import os
    from concourse.bass_utils import run_bass_kernel_spmd

    T = T_FULL
    if "nc" not in _CACHE:
        _CACHE["nc"] = _build_nc(T)
    nc = _CACHE["nc"]
    in_maps = _marshal(inputs, T)
    trace = bool(int(os.environ.get("KERNEL_TRACE", "0")))
    res = run_bass_kernel_spmd(nc, in_maps, core_ids=list(range(NCORES)),
                               trace=trace)
    LAST_RESULTS = res
    out = np.concatenate([res.results[c]["out"] for c in range(NCORES)],
                         axis=0)
    return out.astype(np.float32)
